# revision 17
# baseline (speedup 1.0000x reference)
"""Causal multi-head attention (B=2, S=2048, D=768, H=12) on 8 TRN2 NeuronCores.

Sharding: core c handles batch c//4, heads 3*(c%4) .. 3*(c%4)+3.
Per core (bf16 matmul operands, fp32 PSUM accumulation):
  - q/k projections in transposed layout [hd, S], packed into 3 slots of 128
    partitions per 512-col s-super: [q0|q1], [k0|k1], [q2|k2]; k2 also kept as
    a base-0 partition-shifted copy (PE requires equal operand bases).
  - v projection in natural layout [S, hd] with a ones column appended per
    head (softmax denominator rides along as v_aug column 64).
  - scores computed TRANSPOSED: sT[k, q] = K . Q^T -> exp on ACT -> P^T
  - PV: lhsT = v_aug [k,65], rhs = P^T [k, q] -> ctxT [65, q] accumulated over
    k chunks (row 64 = denominator). Normalize with reciprocal_approx_fast +
    K=1 broadcast matmul. Out-projection: lhsT = Wo^T slices, rhs = ctxT.
  - schedule: x DMA'd sp-major so projections start early; deferred
    projections drip into the ACT-bound attention as PE fillers; softmax
    piece-0 normalization + out-projection are hoisted into the last head's
    chunk loop to shorten the tail; output stores are [128,1024] on HWDGE
    queues.
Host: out[b] = sum of the 4 per-core partial outT^T + bo.
"""

import numpy as np

B, S, D, H, HD = 2, 2048, 768, 12, 64
NH = 3                      # heads per core
NCORES = 8
SCALE = 1.0 / np.sqrt(HD)
QS = 1024                   # q superblock width
NG = S // QS                # 2 q superblocks
NKC = S // 128              # 16 k chunks
NXC = D // 128              # 6 contraction chunks of 128 over D

_cache = {}


def _build(reps=1):
    key = ("nc", reps)
    if key in _cache:
        return _cache[key]
    import concourse.bacc as bacc
    import concourse.mybir as mybir
    import concourse.tile as tile

    f32 = mybir.dt.float32
    bf16 = mybir.dt.bfloat16
    Exp = mybir.ActivationFunctionType.Exp
    add_op = mybir.AluOpType.add

    nc = bacc.Bacc(None, target_bir_lowering=False, debug=False, num_devices=NCORES)

    xT_d = nc.dram_tensor("xT", [D, S], bf16, kind="ExternalInput")
    wqkT_d = nc.dram_tensor("wqkT", [D, 2 * NH * HD], bf16, kind="ExternalInput")
    wvT_d = nc.dram_tensor("wvT", [D, NH * 65], bf16, kind="ExternalInput")
    woT_d = nc.dram_tensor("woT", [128, 2, D], bf16, kind="ExternalInput")
    bqk_d = nc.dram_tensor("bqk", [128, 3], f32, kind="ExternalInput")
    bv_d = nc.dram_tensor("bv", [1, NH * 65], bf16, kind="ExternalInput")
    mask_d = nc.dram_tensor("mask", [128, 128], bf16, kind="ExternalInput")
    outT_d = nc.dram_tensor("outT", [D, S], bf16, kind="ExternalOutput")

    with tile.TileContext(nc) as tc:
        with (
            tc.tile_pool(name="const", bufs=1) as cst,
            tc.tile_pool(name="work", bufs=3) as wrk,
            tc.tile_pool(name="norm", bufs=2) as nrm,
            tc.tile_pool(name="ps_sT", bufs=2, space="PSUM") as ps_sT,
            tc.tile_pool(name="ps_ctx", bufs=1, space="PSUM") as ps_ctx,
            tc.tile_pool(name="ps_mm", bufs=2, space="PSUM") as ps_mm,
        ):
         for _rep in range(reps):
              # ---- DMA plan ----
              # sync (SP HWDGE):  wqk, bqk, x-sp1, x-sp3  (first projection
              #                   inputs + the odd supers)
              # scalar (ACT HWDGE): x-sp0 in 2-chunk pieces (unblocks the very
              #                   first matmuls), then x-sp2
              # gpsimd (SWDGE):   wv, wo, bv, mask (needed later), k2d shifts
              wqk_sb = cst.tile([128, NXC, 2 * NH * HD], bf16)
              wqk_r = wqkT_d[:].rearrange("(c p) m -> p c m", p=128)
              xT_r = xT_d[:].rearrange("(c p) s -> p c s", p=128)
              x_sb = cst.tile([128, NXC, S], bf16)
              bqk_sb = cst.tile([128, 3], f32)
              # interleave the first-needed pieces across the two HWDGE queues
              # so the first projection matmuls start ~1.5us in
              nc.sync.dma_start(wqk_sb[:, 0:3], wqk_r[:, 0:3])
              nc.scalar.dma_start(x_sb[:, 0:2, 0:512], xT_r[:, 0:2, 0:512])
              nc.sync.dma_start(wqk_sb[:, 3:6], wqk_r[:, 3:6])
              nc.scalar.dma_start(x_sb[:, 2:4, 0:512], xT_r[:, 2:4, 0:512])
              nc.sync.dma_start(bqk_sb[:], bqk_d[:])
              nc.scalar.dma_start(x_sb[:, 4:6, 0:512], xT_r[:, 4:6, 0:512])
              nc.sync.dma_start(x_sb[:, :, 512:1024], xT_r[:, :, 512:1024])
              nc.scalar.dma_start(x_sb[:, :, 1024:1536], xT_r[:, :, 1024:1536])
              nc.sync.dma_start(x_sb[:, :, 1536:2048], xT_r[:, :, 1536:2048])

              wv_sb = cst.tile([128, NXC, NH * 65], bf16)
              nc.gpsimd.dma_start(wv_sb[:], wvT_d[:].rearrange("(c p) m -> p c m", p=128))
              wo_sb = cst.tile([128, 2, D], bf16)
              nc.gpsimd.dma_start(wo_sb[:], woT_d[:])
              bv_sb = cst.tile([1, NH * 65], bf16)
              nc.gpsimd.dma_start(bv_sb[:], bv_d[:])
              mask_sb = cst.tile([128, 128], bf16)
              nc.gpsimd.dma_start(mask_sb[:], mask_d[:])

              ones_f = cst.tile([1, 128], f32)
              nc.vector.memset(ones_f[:], 1.0)
              ones_r = cst.tile([1, 128], bf16)
              nc.vector.tensor_copy(ones_r[:], ones_f[:])

              # persistent activations:
              # q/k packed per 512-wide s-super, 3 slots of 128 partitions =
              # [q0|q1], [k0|k1], [q2|k2] (64 partitions per head).
              qk_sb = [
                  cst.tile([128, 3, 512], bf16, tag=f"qksp{sp}", name=f"qksp{sp}")
                  for sp in range(4)
              ]
              # k2 lives at partitions 64:128 of slot 2, but its score matmuls
              # need it at the same base partition as q2 (base 0): keep a
              # partition-shifted copy (local DMA handles the shift).
              k2d = [
                  cst.tile([64, 512], bf16, tag=f"k2d{sp}", name=f"k2d{sp}")
                  for sp in range(4)
              ]
              # v_aug per 128-block: [128, NH, 65]; index 64 is the ones column
              # (written by the v-projection bias matmul via the wvT layout).
              v_sb = [
                  cst.tile([128, NH, 65], bf16, tag=f"vb{b}", name=f"vb{b}")
                  for b in range(NKC)
              ]

              # slots: [q0|q1], [k0|k1], [q2|k2] -> q_h and k_h share a base
              # partition for h=0,1; h=2 uses the k2d shifted copy.
              QK_SLOT = {
                  ("q", 0): (0, 0),
                  ("q", 1): (0, 64),
                  ("k", 0): (1, 0),
                  ("k", 1): (1, 64),
                  ("q", 2): (2, 0),
                  ("k", 2): (2, 64),
              }

              def head_ap(t, h, lo, hi):
                  """AP for head h, global columns [lo, hi) (within one super)."""
                  sp, o = lo // 512, lo % 512
                  if t == "k" and h == 2:
                      return k2d[sp][0:64, o : o + hi - lo]
                  slot, po = QK_SLOT[(t, h)]
                  return qk_sb[sp][po : po + 64, slot, o : o + hi - lo]

              def qk_proj_units(sp, slot):
                  """3 filler quanta of 2 matmuls each (bias add on the last)."""
                  scols = slice(512 * sp, 512 * sp + 512)
                  p = ps_mm.tile([128, 512], f32, tag="mm", name=f"pqk{sp}{slot}")

                  def mm(c0):
                      for c in (c0, c0 + 1):
                          nc.tensor.matmul(
                              p[:],
                              wqk_sb[:, c, 128 * slot : 128 * slot + 128],
                              x_sb[:, c, scols],
                              start=(c == 0),
                              stop=(c == NXC - 1),
                          )
                      if c0 == NXC - 2:
                          nc.vector.tensor_scalar(
                              out=qk_sb[sp][:, slot, :],
                              in0=p[:],
                              scalar1=bqk_sb[:, slot : slot + 1],
                              scalar2=None,
                              op0=add_op,
                          )
                          if slot == 2:
                              nc.gpsimd.dma_start(k2d[sp][:], qk_sb[sp][64:128, 2, :])

                  return [lambda c0=c0: mm(c0) for c0 in range(0, NXC, 2)]

              def qk_proj(sp, slot):
                  for u in qk_proj_units(sp, slot):
                      u()

              def v_proj_units(blk):
                  """2 filler quanta. wvT host layout: col group h*65..h*65+63
                  = head h weights, col h*65+64 = zeros with bias 1.0 -> psum
                  cols [0:195] are the [v_h | 1] groups for all 3 heads."""
                  p = ps_mm.tile(
                      [128, NH * 65], f32, tag="mm", padded_shape=[128, 512],
                      name=f"pv{blk}",
                  )

                  def mm(c0):
                      for c in range(c0, c0 + 3):
                          nc.tensor.matmul(
                              p[:],
                              x_sb[:, c, 128 * blk : 128 * blk + 128],
                              wv_sb[:, c, :],
                              start=(c == 0),
                              stop=False,
                          )
                      if c0 == 3:
                          nc.tensor.matmul(
                              p[:], ones_r[:], bv_sb[:], start=False, stop=True
                          )
                          nc.vector.tensor_copy(v_sb[blk][:, :, :], p[:])

                  return [lambda c0=c0: mm(c0) for c0 in (0, 3)]

              def v_proj(blk):
                  for u in v_proj_units(blk):
                      u()

              # ---- projections needed by superblock g=0 ----
              # ordered to match DMA arrival: sp0 cols land first (v blocks
              # 0-3 read only sp0 cols), then sp1.
              for slot in range(3):
                  qk_proj(0, slot)
              for blk in range(4):
                  v_proj(blk)
              for slot in range(3):
                  qk_proj(1, slot)
              for blk in range(4, 8):
                  v_proj(blk)

              # Deferred work drips into the attention loops as one small PE
              # quantum (~0.4-0.6us) per chunk iteration, keeping the PE fed
              # while ACT runs exp. One unit is popped per chunk; out-
              # projection units are appended once their ctn inputs resolve.
              # Ordering constraints: sp2/sp3 q slots (0 and 2) before g=1
              # starts (iteration 24); k slots (1) before g=1 reaches chunk
              # 8 (iter 32) / 12 (iter 36); v block b before iter 24+b.
              opq = []
              fillers = (
                  qk_proj_units(2, 0)
                  + qk_proj_units(3, 0)
                  + qk_proj_units(2, 2)
                  + qk_proj_units(3, 2)
                  + v_proj_units(8)
                  + v_proj_units(9)
                  + qk_proj_units(2, 1)
                  + v_proj_units(10)
                  + v_proj_units(11)
                  + qk_proj_units(3, 1)
                  + v_proj_units(12)
                  + v_proj_units(13)
                  + v_proj_units(14)
                  + v_proj_units(15)
              )

              def norm_piece(g, h, ctx, ctn, piece):
                  """ctx piece -> normalized bf16 ctn slice (one 512 window).

                  reciprocal reads the denominator row straight from PSUM;
                  the K=1 ones matmul broadcasts it across 64 partitions.
                  """
                  pcols = slice(512 * piece, 512 * piece + 512)
                  den = nrm.tile([1, 512], f32, tag=f"den{piece}", name=f"den{piece}")
                  nc.scalar.copy(den[:], ctx[64:65, pcols])
                  rec = nrm.tile(
                      [1, 512], f32, tag=f"rec{piece}", name=f"rec{piece}"
                  )
                  nc.vector.reciprocal_approx_fast(out=rec[:], in_=den[:])
                  recr = nrm.tile(
                      [1, 512], bf16, tag=f"recr{piece}", name=f"recr{piece}"
                  )
                  nc.vector.tensor_copy(recr[:], rec[:])
                  bc = ps_mm.tile([64, 512], f32, tag="mm")
                  nc.tensor.matmul(bc[:], ones_r[:, 0:64], recr[:], start=True, stop=True)
                  cts = nrm.tile([64, 512], f32, tag=f"cts{piece}", name=f"cts{piece}")
                  nc.vector.tensor_copy(cts[:], ctx[0:64, pcols])
                  dst = (
                      ctn[piece][64 * h : 64 * h + 64, 0, :]
                      if h < 2
                      else ctn[piece][0:64, 1, :]
                  )
                  nc.vector.tensor_mul(dst, cts[:], bc[:])

              def outproj_unit(g, ctn, piece, ot_tiles, jc):
                  """One row block of one 512-col piece of the out-projection."""
                  po = ps_mm.tile([128, 512], f32, tag="mm", name=f"po{jc}")
                  nc.tensor.matmul(
                      po[:],
                      wo_sb[:, 0, 128 * jc : 128 * jc + 128],
                      ctn[piece][:, 0, :],
                      start=True,
                      stop=False,
                  )
                  nc.tensor.matmul(
                      po[:],
                      wo_sb[0:64, 1, 128 * jc : 128 * jc + 128],
                      ctn[piece][0:64, 1, :],
                      start=False,
                      stop=True,
                  )
                  ot = ot_tiles[jc]
                  nc.vector.tensor_copy(ot[:, 512 * piece : 512 * piece + 512], po[:])
                  if piece == 1:
                      eng = nc.sync if jc % 2 == 0 else nc.scalar
                      eng.dma_start(
                          outT_d[128 * jc : 128 * jc + 128, QS * g : QS * g + QS],
                          ot[:],
                      )

              def outproj_units(g, ctn, piece, ot_tiles):
                  return [
                      lambda jc=jc: outproj_unit(g, ctn, piece, ot_tiles, jc)
                      for jc in range(6)
                  ]

              # ---- attention + out-projection per q superblock ----
              for g in range(NG):
                  # normalized ctxT per 512-piece (finer outproj deps)
                  # packed: [0:64,0]=h0, [64:128,0]=h1, [0:64,1]=h2
                  ctn = [
                      nrm.tile([128, 2, 512], bf16, tag=f"ctn{p}", name=f"ctn{p}_{g}")
                      for p in range(2)
                  ]
                  ot_tiles = [
                      wrk.tile(
                          [128, QS], bf16, tag=f"ot{jc}", name=f"ot{jc}_{g}", bufs=2
                      )
                      for jc in range(6)
                  ]
                  for h in range(NH):
                      ctx = ps_ctx.tile([65, QS], f32)
                      nchunks = 8 * g + 8

                      def emit_pv(c, pt):
                          """PV for chunk c (one iteration behind the score/exp
                          issue so the exp latency never stalls the PE)."""
                          j = c - 8 * g
                          q0 = max(0, 128 * j)
                          for piece in range(2):
                              lo, hi = max(q0, 512 * piece), 512 * piece + 512
                              if lo >= hi:
                                  continue
                              nc.tensor.matmul(
                                  ctx[:, lo:hi],
                                  v_sb[c][:, h, :],
                                  pt[:, lo:hi],
                                  start=(c == 0),
                                  stop=(c == nchunks - 1 or (piece == 0 and j >= 3)),
                              )
                          # piece 0 of ctx is final once the diagonal passes
                          # column 512 (j == 3): normalize it now so the
                          # out-projection can start before the head finishes.
                          if j == 3:
                              norm_piece(g, h, ctx, ctn, 0)
                              if h == NH - 1:
                                  opq.extend(outproj_units(g, ctn, 0, ot_tiles))

                      prev = None
                      for c in range(nchunks):
                          j = c - 8 * g  # >=0 inside the diagonal region
                          q0 = max(0, 128 * j)  # valid q start (rel. to super)
                          sT = ps_sT.tile([128, QS], f32)
                          for piece in range(2):
                              lo, hi = max(q0, 512 * piece), 512 * piece + 512
                              if lo >= hi:
                                  continue
                              nc.tensor.matmul(
                                  sT[:, lo:hi],
                                  head_ap("k", h, 128 * c, 128 * c + 128),
                                  head_ap("q", h, QS * g + lo, QS * g + hi),
                                  start=True,
                                  stop=True,
                              )
                          pt = wrk.tile([128, QS], bf16, tag="pt")
                          nc.scalar.activation(
                              pt[:, q0:QS], sT[:, q0:QS], Exp, scale=float(SCALE)
                          )
                          if j >= 0:
                              # SBUF-only elementwise -> offload to idle GpSimd
                              nc.gpsimd.tensor_mul(
                                  pt[:, q0 : q0 + 128],
                                  pt[:, q0 : q0 + 128],
                                  mask_sb[:],
                              )
                          if fillers:
                              fillers.pop(0)()
                          elif opq and c % 2 == 0:
                              # out-projection units pace at half rate so the
                              # queue lasts deep into the ACT-bound stretch
                              opq.pop(0)()
                          if prev is not None:
                              emit_pv(*prev)
                          prev = (c, pt)
                      emit_pv(*prev)
                      norm_piece(g, h, ctx, ctn, 1)
                  # drain before this superblock's final out-projection piece
                  while fillers:
                      fillers.pop(0)()
                  if g == NG - 1:
                      while opq:
                          opq.pop(0)()
                  opq.extend(outproj_units(g, ctn, 1, ot_tiles))
              while opq:
                  opq.pop(0)()

    nc.compile()
    _cache[key] = nc
    return nc


def kernel(x, Wq, bq, Wk, bk, Wv, bv, Wo, bo):
    out, _ = run(x, Wq, bq, Wk, bk, Wv, bv, Wo, bo)
    return out


def build_in_maps(x, Wq, bq, Wk, bk, Wv, bv, Wo, bo=None):
    from ml_dtypes import bfloat16

    x = np.asarray(x, np.float32)
    Wq, bq = np.asarray(Wq, np.float32), np.asarray(bq, np.float32)
    Wk, bk = np.asarray(Wk, np.float32), np.asarray(bk, np.float32)
    Wv, bv = np.asarray(Wv, np.float32), np.asarray(bv, np.float32)
    Wo = np.asarray(Wo, np.float32)

    mask = np.triu(np.ones((128, 128), bfloat16))  # [k_l, q_l]: 1 where q_l >= k_l
    in_maps = []
    for c in range(NCORES):
        b, rs = c // 4, (c % 4) * NH * HD
        re = rs + NH * HD
        # per-head [64 weight cols | 1 zero col] groups; bias row carries the
        # head biases and a 1.0 in each group's last column (the ones column).
        woP = np.zeros((128, 2, D), np.float32)
        woP[:, 0, :] = Wo[:, rs : rs + 128].T
        woP[0:64, 1, :] = Wo[:, rs + 128 : rs + 192].T
        wvT = np.zeros((D, 195), np.float32)
        bv_row = np.zeros((1, 195), np.float32)
        for h in range(NH):
            wvT[:, 65 * h : 65 * h + 64] = Wv[rs + 64 * h : rs + 64 * h + 64].T
            bv_row[0, 65 * h : 65 * h + 64] = bv[rs + 64 * h : rs + 64 * h + 64]
            bv_row[0, 65 * h + 64] = 1.0
        # packed q/k slots: [q0|q1], [k0|k1], [q2|k2] (128 output cols each)
        wqkT = np.concatenate(
            [
                Wq[rs : rs + 128].T,
                Wk[rs : rs + 128].T,
                Wq[rs + 128 : re].T,
                Wk[rs + 128 : re].T,
            ],
            axis=1,
        )
        bqk = np.stack(
            [
                bq[rs : rs + 128],
                bk[rs : rs + 128],
                np.concatenate([bq[rs + 128 : re], bk[rs + 128 : re]]),
            ],
            axis=1,
        )
        in_maps.append(
            {
                "xT": np.ascontiguousarray(x[b].T).astype(bfloat16),
                "wqkT": np.ascontiguousarray(wqkT).astype(bfloat16),
                "wvT": wvT.astype(bfloat16),
                "woT": woP.astype(bfloat16),
                "bqk": np.ascontiguousarray(bqk, np.float32),
                "bv": bv_row.astype(bfloat16),
                "mask": mask,
            }
        )
    return in_maps


def run(x, Wq, bq, Wk, bk, Wv, bv, Wo, bo, trace=False):
    from concourse.bass_utils import run_bass_kernel_spmd

    nc = _build()
    bo = np.asarray(bo, np.float32)
    in_maps = build_in_maps(x, Wq, bq, Wk, bk, Wv, bv, Wo)
    res = run_bass_kernel_spmd(nc, in_maps, list(range(NCORES)), trace=trace)
    out = np.zeros((B, S, D), np.float32)
    for b in range(B):
        acc = np.zeros((D, S), np.float32)
        for c in range(4 * b, 4 * b + 4):
            acc += res.results[c]["outT"].astype(np.float32)
        out[b] = acc.T + bo
    return out, res


# revision 19
# speedup vs baseline: 2.1531x; 2.1531x over previous
"""Causal multi-head attention (B=2, S=2048, D=768, H=12) on 8 TRN2 NeuronCores.

Sharding: core c handles batch c//4, heads 3*(c%4) .. 3*(c%4)+3.
Per core (bf16 matmul operands, fp32 PSUM accumulation):
  - q/k projections in transposed layout [hd, S], packed into 3 slots of 128
    partitions per 512-col s-super: [q0|q1], [k0|k1], [q2|k2]; k2 also kept as
    a base-0 partition-shifted copy (PE requires equal operand bases).
  - v projection in natural layout [S, hd] with a ones column appended per
    head (softmax denominator rides along as v_aug column 64).
  - scores computed TRANSPOSED: sT[k, q] = K . Q^T -> exp on ACT -> P^T
  - PV: lhsT = v_aug [k,65], rhs = P^T [k, q] -> ctxT [65, q] accumulated over
    k chunks (row 64 = denominator). Normalize with reciprocal_approx_fast +
    K=1 broadcast matmul. Out-projection: lhsT = Wo^T slices, rhs = ctxT.
  - schedule: x DMA'd sp-major so projections start early; deferred
    projections drip into the ACT-bound attention as PE fillers; softmax
    piece-0 normalization + out-projection are hoisted into the last head's
    chunk loop to shorten the tail; output stores are [128,1024] on HWDGE
    queues.
Host: out[b] = sum of the 4 per-core partial outT^T + bo.
"""

import numpy as np

B, S, D, H, HD = 2, 2048, 768, 12, 64
NH = 3                      # heads per core
NCORES = 8
SCALE = 1.0 / np.sqrt(HD)
QS = 1024                   # q superblock width
NG = S // QS                # 2 q superblocks
NKC = S // 128              # 16 k chunks
NXC = D // 128              # 6 contraction chunks of 128 over D

_cache = {}


def _build(reps=1):
    key = ("nc", reps)
    if key in _cache:
        return _cache[key]
    import concourse.bacc as bacc
    import concourse.mybir as mybir
    import concourse.tile as tile

    f32 = mybir.dt.float32
    bf16 = mybir.dt.bfloat16
    Exp = mybir.ActivationFunctionType.Exp
    add_op = mybir.AluOpType.add

    nc = bacc.Bacc(None, target_bir_lowering=False, debug=False, num_devices=NCORES)

    xT_d = nc.dram_tensor("xT", [D, S], bf16, kind="ExternalInput")
    wqkT_d = nc.dram_tensor("wqkT", [D, 2 * NH * HD], bf16, kind="ExternalInput")
    wvT_d = nc.dram_tensor("wvT", [D, NH * 65], bf16, kind="ExternalInput")
    woT_d = nc.dram_tensor("woT", [128, 2, D], bf16, kind="ExternalInput")
    bqk_d = nc.dram_tensor("bqk", [128, 3], f32, kind="ExternalInput")
    bv_d = nc.dram_tensor("bv", [1, NH * 65], bf16, kind="ExternalInput")
    mask_d = nc.dram_tensor("mask", [128, 128], bf16, kind="ExternalInput")
    outT_d = nc.dram_tensor("outT", [D, S], bf16, kind="ExternalOutput")

    with tile.TileContext(nc) as tc:
        with (
            tc.tile_pool(name="const", bufs=2) as cst,
            tc.tile_pool(name="work", bufs=3) as wrk,
            tc.tile_pool(name="norm", bufs=2) as nrm,
            tc.tile_pool(name="ps_sT", bufs=2, space="PSUM") as ps_sT,
            tc.tile_pool(name="ps_ctx", bufs=1, space="PSUM") as ps_ctx,
            tc.tile_pool(name="ps_mm", bufs=2, space="PSUM") as ps_mm,
        ):
         for _rep in range(reps):
              # ---- DMA plan ----
              # sync (SP HWDGE):  wqk, bqk, x-sp1, x-sp3  (first projection
              #                   inputs + the odd supers)
              # scalar (ACT HWDGE): x-sp0 in 2-chunk pieces (unblocks the very
              #                   first matmuls), then x-sp2
              # gpsimd (SWDGE):   wv, wo, bv, mask (needed later), k2d shifts
              wqk_sb = cst.tile([128, NXC, 2 * NH * HD], bf16, tag="wqk")
              wqk_r = wqkT_d[:].rearrange("(c p) m -> p c m", p=128)
              xT_r = xT_d[:].rearrange("(c p) s -> p c s", p=128)
              x_sb = cst.tile([128, NXC, S], bf16, tag="x")
              bqk_sb = cst.tile([128, 3], f32, tag="bqk")
              # interleave the first-needed pieces across the two HWDGE queues
              # so the first projection matmuls start ~1.5us in
              nc.sync.dma_start(wqk_sb[:, 0:3], wqk_r[:, 0:3])
              nc.scalar.dma_start(x_sb[:, 0:2, 0:512], xT_r[:, 0:2, 0:512])
              nc.sync.dma_start(wqk_sb[:, 3:6], wqk_r[:, 3:6])
              nc.scalar.dma_start(x_sb[:, 2:4, 0:512], xT_r[:, 2:4, 0:512])
              nc.sync.dma_start(bqk_sb[:], bqk_d[:])
              nc.scalar.dma_start(x_sb[:, 4:6, 0:512], xT_r[:, 4:6, 0:512])
              nc.sync.dma_start(x_sb[:, :, 512:1024], xT_r[:, :, 512:1024])
              nc.scalar.dma_start(x_sb[:, :, 1024:1536], xT_r[:, :, 1024:1536])
              nc.sync.dma_start(x_sb[:, :, 1536:2048], xT_r[:, :, 1536:2048])

              wv_sb = cst.tile([128, NXC, NH * 65], bf16, tag="wv")
              nc.gpsimd.dma_start(wv_sb[:], wvT_d[:].rearrange("(c p) m -> p c m", p=128))
              wo_sb = cst.tile([128, 2, D], bf16, tag="wo")
              nc.gpsimd.dma_start(wo_sb[:], woT_d[:])
              bv_sb = cst.tile([1, NH * 65], bf16, tag="bv")
              nc.gpsimd.dma_start(bv_sb[:], bv_d[:])
              mask_sb = cst.tile([128, 128], bf16, tag="mask")
              nc.gpsimd.dma_start(mask_sb[:], mask_d[:])

              ones_f = cst.tile([1, 128], f32, tag="ones_f")
              nc.vector.memset(ones_f[:], 1.0)
              ones_r = cst.tile([1, 128], bf16, tag="ones_r")
              nc.vector.tensor_copy(ones_r[:], ones_f[:])

              # persistent activations:
              # q/k packed per 512-wide s-super, 3 slots of 128 partitions =
              # [q0|q1], [k0|k1], [q2|k2] (64 partitions per head).
              qk_sb = [
                  cst.tile([128, 3, 512], bf16, tag=f"qksp{sp}", name=f"qksp{sp}")
                  for sp in range(4)
              ]
              # k2 lives at partitions 64:128 of slot 2, but its score matmuls
              # need it at the same base partition as q2 (base 0): keep a
              # partition-shifted copy (local DMA handles the shift).
              k2d = [
                  cst.tile([64, 512], bf16, tag=f"k2d{sp}", name=f"k2d{sp}")
                  for sp in range(4)
              ]
              # v_aug per 128-block: [128, NH, 65]; index 64 is the ones column
              # (written by the v-projection bias matmul via the wvT layout).
              v_sb = [
                  cst.tile([128, NH, 65], bf16, tag=f"vb{b}", name=f"vb{b}")
                  for b in range(NKC)
              ]

              # slots: [q0|q1], [k0|k1], [q2|k2] -> q_h and k_h share a base
              # partition for h=0,1; h=2 uses the k2d shifted copy.
              QK_SLOT = {
                  ("q", 0): (0, 0),
                  ("q", 1): (0, 64),
                  ("k", 0): (1, 0),
                  ("k", 1): (1, 64),
                  ("q", 2): (2, 0),
                  ("k", 2): (2, 64),
              }

              def head_ap(t, h, lo, hi):
                  """AP for head h, global columns [lo, hi) (within one super)."""
                  sp, o = lo // 512, lo % 512
                  if t == "k" and h == 2:
                      return k2d[sp][0:64, o : o + hi - lo]
                  slot, po = QK_SLOT[(t, h)]
                  return qk_sb[sp][po : po + 64, slot, o : o + hi - lo]

              def qk_proj_units(sp, slot):
                  """3 filler quanta of 2 matmuls each (bias add on the last)."""
                  scols = slice(512 * sp, 512 * sp + 512)
                  p = ps_mm.tile([128, 512], f32, tag="mm", name=f"pqk{sp}{slot}")

                  def mm(c0):
                      for c in (c0, c0 + 1):
                          nc.tensor.matmul(
                              p[:],
                              wqk_sb[:, c, 128 * slot : 128 * slot + 128],
                              x_sb[:, c, scols],
                              start=(c == 0),
                              stop=(c == NXC - 1),
                          )
                      if c0 == NXC - 2:
                          nc.vector.tensor_scalar(
                              out=qk_sb[sp][:, slot, :],
                              in0=p[:],
                              scalar1=bqk_sb[:, slot : slot + 1],
                              scalar2=None,
                              op0=add_op,
                          )
                          if slot == 2:
                              nc.gpsimd.dma_start(k2d[sp][:], qk_sb[sp][64:128, 2, :])

                  return [lambda c0=c0: mm(c0) for c0 in range(0, NXC, 2)]

              def qk_proj(sp, slot):
                  for u in qk_proj_units(sp, slot):
                      u()

              def v_proj_units(blk):
                  """2 filler quanta. wvT host layout: col group h*65..h*65+63
                  = head h weights, col h*65+64 = zeros with bias 1.0 -> psum
                  cols [0:195] are the [v_h | 1] groups for all 3 heads."""
                  p = ps_mm.tile(
                      [128, NH * 65], f32, tag="mm", padded_shape=[128, 512],
                      name=f"pv{blk}",
                  )

                  def mm(c0):
                      for c in range(c0, c0 + 3):
                          nc.tensor.matmul(
                              p[:],
                              x_sb[:, c, 128 * blk : 128 * blk + 128],
                              wv_sb[:, c, :],
                              start=(c == 0),
                              stop=False,
                          )
                      if c0 == 3:
                          nc.tensor.matmul(
                              p[:], ones_r[:], bv_sb[:], start=False, stop=True
                          )
                          nc.vector.tensor_copy(v_sb[blk][:, :, :], p[:])

                  return [lambda c0=c0: mm(c0) for c0 in (0, 3)]

              def v_proj(blk):
                  for u in v_proj_units(blk):
                      u()

              # ---- projections needed by superblock g=0 ----
              # ordered to match DMA arrival: sp0 cols land first (v blocks
              # 0-3 read only sp0 cols), then sp1.
              for slot in range(3):
                  qk_proj(0, slot)
              for blk in range(4):
                  v_proj(blk)
              for slot in range(3):
                  qk_proj(1, slot)
              for blk in range(4, 8):
                  v_proj(blk)

              # Deferred work drips into the attention loops as one small PE
              # quantum (~0.4-0.6us) per chunk iteration, keeping the PE fed
              # while ACT runs exp. One unit is popped per chunk; out-
              # projection units are appended once their ctn inputs resolve.
              # Ordering constraints: sp2/sp3 q slots (0 and 2) before g=1
              # starts (iteration 24); k slots (1) before g=1 reaches chunk
              # 8 (iter 32) / 12 (iter 36); v block b before iter 24+b.
              opq = []
              fillers = (
                  qk_proj_units(2, 0)
                  + qk_proj_units(3, 0)
                  + qk_proj_units(2, 2)
                  + qk_proj_units(3, 2)
                  + v_proj_units(8)
                  + v_proj_units(9)
                  + qk_proj_units(2, 1)
                  + v_proj_units(10)
                  + v_proj_units(11)
                  + qk_proj_units(3, 1)
                  + v_proj_units(12)
                  + v_proj_units(13)
                  + v_proj_units(14)
                  + v_proj_units(15)
              )

              def norm_span(g, h, ctx, ctn, p0, np_):
                  """Normalize ctx cols [512*p0, 512*(p0+np_)) into bf16 ctn.

                  The denominator row is staged PSUM->SBUF on ACT (custom DVE
                  ISA ops cannot read PSUM on hardware), reciprocal'd on DVE,
                  then broadcast across 64 partitions by a K=1 ones matmul.
                  Scalar chain ops run np_*512 wide; the bc matmul and the
                  ctn mul stay per-512 (PSUM bank / ctn tile granularity).
                  """
                  w = 512 * np_
                  pcols = slice(512 * p0, 512 * p0 + w)
                  den = nrm.tile([1, w], f32, tag=f"den{p0}{np_}", name=f"den{p0}")
                  nc.scalar.copy(den[:], ctx[64:65, pcols])
                  rec = nrm.tile([1, w], f32, tag=f"rec{p0}{np_}", name=f"rec{p0}")
                  nc.vector.reciprocal_approx_fast(out=rec[:], in_=den[:])
                  recr = nrm.tile([1, w], bf16, tag=f"recr{p0}{np_}", name=f"recr{p0}")
                  nc.vector.tensor_copy(recr[:], rec[:])
                  cts = nrm.tile([64, w], f32, tag=f"cts{p0}{np_}", name=f"cts{p0}")
                  nc.vector.tensor_copy(cts[:], ctx[0:64, pcols])
                  for piece in range(p0, p0 + np_):
                      o = slice(512 * (piece - p0), 512 * (piece - p0) + 512)
                      bc = ps_mm.tile([64, 512], f32, tag="mm")
                      nc.tensor.matmul(
                          bc[:], ones_r[:, 0:64], recr[:, o], start=True, stop=True
                      )
                      dst = (
                          ctn[piece][64 * h : 64 * h + 64, 0, :]
                          if h < 2
                          else ctn[piece][0:64, 1, :]
                      )
                      nc.vector.tensor_mul(dst, cts[:, o], bc[:])

              def norm_piece(g, h, ctx, ctn, piece):
                  norm_span(g, h, ctx, ctn, piece, 1)

              def outproj_unit(g, ctn, piece, ot_tiles, jc):
                  """One row block of one 512-col piece of the out-projection."""
                  po = ps_mm.tile([128, 512], f32, tag="mm", name=f"po{jc}")
                  nc.tensor.matmul(
                      po[:],
                      wo_sb[:, 0, 128 * jc : 128 * jc + 128],
                      ctn[piece][:, 0, :],
                      start=True,
                      stop=False,
                  )
                  nc.tensor.matmul(
                      po[:],
                      wo_sb[0:64, 1, 128 * jc : 128 * jc + 128],
                      ctn[piece][0:64, 1, :],
                      start=False,
                      stop=True,
                  )
                  ot = ot_tiles[jc]
                  nc.vector.tensor_copy(ot[:, 512 * piece : 512 * piece + 512], po[:])
                  if piece == 1:
                      eng = nc.sync if jc % 2 == 0 else nc.scalar
                      eng.dma_start(
                          outT_d[128 * jc : 128 * jc + 128, QS * g : QS * g + QS],
                          ot[:],
                      )

              def outproj_units(g, ctn, piece, ot_tiles):
                  return [
                      lambda jc=jc: outproj_unit(g, ctn, piece, ot_tiles, jc)
                      for jc in range(6)
                  ]

              # ---- attention + out-projection per q superblock ----
              for g in range(NG):
                  # normalized ctxT per 512-piece (finer outproj deps)
                  # packed: [0:64,0]=h0, [64:128,0]=h1, [0:64,1]=h2
                  ctn = [
                      nrm.tile([128, 2, 512], bf16, tag=f"ctn{p}", name=f"ctn{p}_{g}")
                      for p in range(2)
                  ]
                  ot_tiles = [
                      wrk.tile(
                          [128, QS], bf16, tag=f"ot{jc}", name=f"ot{jc}_{g}", bufs=2
                      )
                      for jc in range(6)
                  ]
                  for h in range(NH):
                      ctx = ps_ctx.tile([65, QS], f32)
                      nchunks = 8 * g + 8

                      def emit_pv(c, pt):
                          """PV for chunk c (one iteration behind the score/exp
                          issue so the exp latency never stalls the PE)."""
                          j = c - 8 * g
                          q0 = max(0, 128 * j)
                          for piece in range(2):
                              lo, hi = max(q0, 512 * piece), 512 * piece + 512
                              if lo >= hi:
                                  continue
                              nc.tensor.matmul(
                                  ctx[:, lo:hi],
                                  v_sb[c][:, h, :],
                                  pt[:, lo:hi],
                                  start=(c == 0),
                                  stop=(c == nchunks - 1 or (piece == 0 and j >= 3)),
                              )
                          # piece 0 of ctx is final once the diagonal passes
                          # column 512 (j == 3): normalize it now so the
                          # out-projection can start before the head finishes.
                          if j == 3 and h == NH - 1:
                              norm_piece(g, h, ctx, ctn, 0)
                              opq.extend(outproj_units(g, ctn, 0, ot_tiles))

                      prev = None
                      for c in range(nchunks):
                          j = c - 8 * g  # >=0 inside the diagonal region
                          q0 = max(0, 128 * j)  # valid q start (rel. to super)
                          sT = ps_sT.tile([128, QS], f32)
                          for piece in range(2):
                              lo, hi = max(q0, 512 * piece), 512 * piece + 512
                              if lo >= hi:
                                  continue
                              nc.tensor.matmul(
                                  sT[:, lo:hi],
                                  head_ap("k", h, 128 * c, 128 * c + 128),
                                  head_ap("q", h, QS * g + lo, QS * g + hi),
                                  start=True,
                                  stop=True,
                              )
                          pt = wrk.tile([128, QS], bf16, tag="pt")
                          nc.scalar.activation(
                              pt[:, q0:QS], sT[:, q0:QS], Exp, scale=float(SCALE)
                          )
                          if j >= 0:
                              # SBUF-only elementwise -> offload to idle GpSimd
                              nc.gpsimd.tensor_mul(
                                  pt[:, q0 : q0 + 128],
                                  pt[:, q0 : q0 + 128],
                                  mask_sb[:],
                              )
                          if fillers:
                              fillers.pop(0)()
                          elif opq and c % 2 == 0:
                              # out-projection units pace at half rate so the
                              # queue lasts deep into the ACT-bound stretch
                              opq.pop(0)()
                          if prev is not None:
                              emit_pv(*prev)
                          prev = (c, pt)
                      emit_pv(*prev)
                      if h < NH - 1:
                          norm_span(g, h, ctx, ctn, 0, 2)
                      else:
                          norm_piece(g, h, ctx, ctn, 1)
                  # drain before this superblock's final out-projection piece
                  while fillers:
                      fillers.pop(0)()
                  if g == NG - 1:
                      while opq:
                          opq.pop(0)()
                  opq.extend(outproj_units(g, ctn, 1, ot_tiles))
              while opq:
                  opq.pop(0)()

    nc.compile()
    _cache[key] = nc
    return nc


def kernel(x, Wq, bq, Wk, bk, Wv, bv, Wo, bo):
    out, _ = run(x, Wq, bq, Wk, bk, Wv, bv, Wo, bo)
    return out


def build_in_maps(x, Wq, bq, Wk, bk, Wv, bv, Wo, bo=None):
    from ml_dtypes import bfloat16

    x = np.asarray(x, np.float32)
    Wq, bq = np.asarray(Wq, np.float32), np.asarray(bq, np.float32)
    Wk, bk = np.asarray(Wk, np.float32), np.asarray(bk, np.float32)
    Wv, bv = np.asarray(Wv, np.float32), np.asarray(bv, np.float32)
    Wo = np.asarray(Wo, np.float32)

    mask = np.triu(np.ones((128, 128), bfloat16))  # [k_l, q_l]: 1 where q_l >= k_l
    in_maps = []
    for c in range(NCORES):
        b, rs = c // 4, (c % 4) * NH * HD
        re = rs + NH * HD
        # per-head [64 weight cols | 1 zero col] groups; bias row carries the
        # head biases and a 1.0 in each group's last column (the ones column).
        woP = np.zeros((128, 2, D), np.float32)
        woP[:, 0, :] = Wo[:, rs : rs + 128].T
        woP[0:64, 1, :] = Wo[:, rs + 128 : rs + 192].T
        wvT = np.zeros((D, 195), np.float32)
        bv_row = np.zeros((1, 195), np.float32)
        for h in range(NH):
            wvT[:, 65 * h : 65 * h + 64] = Wv[rs + 64 * h : rs + 64 * h + 64].T
            bv_row[0, 65 * h : 65 * h + 64] = bv[rs + 64 * h : rs + 64 * h + 64]
            bv_row[0, 65 * h + 64] = 1.0
        # packed q/k slots: [q0|q1], [k0|k1], [q2|k2] (128 output cols each)
        wqkT = np.concatenate(
            [
                Wq[rs : rs + 128].T,
                Wk[rs : rs + 128].T,
                Wq[rs + 128 : re].T,
                Wk[rs + 128 : re].T,
            ],
            axis=1,
        )
        bqk = np.stack(
            [
                bq[rs : rs + 128],
                bk[rs : rs + 128],
                np.concatenate([bq[rs + 128 : re], bk[rs + 128 : re]]),
            ],
            axis=1,
        )
        in_maps.append(
            {
                "xT": np.ascontiguousarray(x[b].T).astype(bfloat16),
                "wqkT": np.ascontiguousarray(wqkT).astype(bfloat16),
                "wvT": wvT.astype(bfloat16),
                "woT": woP.astype(bfloat16),
                "bqk": np.ascontiguousarray(bqk, np.float32),
                "bv": bv_row.astype(bfloat16),
                "mask": mask,
            }
        )
    return in_maps


def run(x, Wq, bq, Wk, bk, Wv, bv, Wo, bo, trace=False):
    from concourse.bass_utils import run_bass_kernel_spmd

    nc = _build()
    bo = np.asarray(bo, np.float32)
    in_maps = build_in_maps(x, Wq, bq, Wk, bk, Wv, bv, Wo)
    res = run_bass_kernel_spmd(nc, in_maps, list(range(NCORES)), trace=trace)
    out = np.zeros((B, S, D), np.float32)
    for b in range(B):
        acc = np.zeros((D, S), np.float32)
        for c in range(4 * b, 4 * b + 4):
            acc += res.results[c]["outT"].astype(np.float32)
        out[b] = acc.T + bo
    return out, res
